# revision 20
# baseline (speedup 1.0000x reference)
"""AttentionSubsample kernel for 8 Trainium2 NeuronCores.

Sharding: 8 cores = 4 batches x 2 sequence-halves. Each core computes
attention for its 2048 queries against the batch's 1024 subsampled keys
(K/V projections are computed per-core from the strided token subset, so
only ~querylen/stride extra work is duplicated across the 2 cores that
share a batch).

Matmuls run in bf16 (fp32 PSUM accumulation). Inputs are cast to bf16 on
the host, so every matmul operand can be loaded straight from DRAM with
the DMA crossbar transpose (the contraction dim must sit on SBUF
partitions, and both x and the weights store it contiguous-last).
"""

import sys

sys.path.insert(0, "/opt/trn_rl_repo")

import numpy as np
import ml_dtypes

import concourse.bacc as bacc
import concourse.bass as bass
import concourse.mybir as mybir
import concourse.tile as tile
from concourse import bass_utils
from concourse.bass import ts

F32 = mybir.dt.float32
BF16 = mybir.dt.bfloat16
EXP = mybir.ActivationFunctionType.Exp

N_CORES = 8
DEBUG = False
_cache = {}


def _build(S_loc, K, D, H, scale):
    """Build the per-core Bass program. S_loc queries, K keys, D model dim."""
    dh = D // H          # 64 head dim
    nP = D // 128        # 6 head pairs / contraction blocks
    nKB = K // 128       # 8 key blocks
    QCW = 1024           # q-chunk width for attention
    nQC = S_loc // QCW   # 2
    assert dh == 64 and D % 128 == 0 and K % 128 == 0 and S_loc % QCW == 0

    nc = bacc.Bacc("TRN2", target_bir_lowering=False, debug=False,
                   num_devices=N_CORES)

    xq_d = nc.dram_tensor("xq", [S_loc, D], BF16, kind="ExternalInput").ap()
    xkv_d = nc.dram_tensor("xkv", [K, D], BF16, kind="ExternalInput").ap()
    w_d = nc.dram_tensor("qkv_w", [3 * D, D], BF16, kind="ExternalInput").ap()
    pw_d = nc.dram_tensor("proj_w", [D, D], BF16, kind="ExternalInput").ap()
    pb_d = nc.dram_tensor("proj_b", [1, D], F32, kind="ExternalInput").ap()
    y_d = nc.dram_tensor("y", [S_loc, D], F32, kind="ExternalOutput").ap()
    dbg = {}
    if DEBUG:
        for name, shape in [("kT0", [128, K]), ("vaug0", [128, H * (dh + 1)]),
                            ("qT0", [128, QCW]), ("ex00", [128, QCW]),
                            ("av00", [dh + 1, 512]), ("bc00", [dh, 512]),
                            ("outT0", [128, QCW])]:
            dbg[name] = nc.dram_tensor(name, shape, F32,
                                       kind="ExternalOutput").ap()

    with tile.TileContext(nc, pool_alloc_mode="queue") as tc:
        ctx_pools = []

        def pool(name, bufs, space="SBUF"):
            p = tc.alloc_tile_pool(name=name, bufs=bufs, space=space)
            ctx_pools.append(p)
            return p

        try:
            _build_body(nc, tc, pool, xq_d, xkv_d, w_d, pw_d, pb_d, y_d,
                        S_loc, K, D, H, dh, nP, nKB, QCW, nQC, scale, dbg)
        finally:
            for p in reversed(ctx_pools):
                if not p._released:
                    p.release()

    nc.compile()
    return nc


def _build_body(nc, tc, pool, xq_d, xkv_d, w_d, pw_d, pb_d, y_d,
                S_loc, K, D, H, dh, nP, nKB, QCW, nQC, scale, dbg={}):
    misc = pool("misc", 1)
    stage = pool("stage", 3)
    psum = pool("psum", 1, space="PSUM")

    def dump(name, ap):
        if name not in dbg:
            return
        t = stage.tile(list(ap.shape), F32, name=f"d_{name}", tag=f"d_{name}",
                       bufs=1)
        nc.vector.tensor_copy(t[:], ap)
        nc.sync.dma_start(dbg[name][:], t[:])

    def tload(out_ap, in_ap):
        """Transposed load (bf16 DMA crossbar). Sync queue only: the
        SBUF-write crossbar transpose mode races copies from other DMA
        queues (known HW bug)."""
        nc.sync.dma_start(out_ap, in_ap, transpose=True)

    # ---- V projection: v_aug[kb] [128, H*(dh+1)] with ones col per head ----
    xkvT_pool = pool("xkvT", 1)
    xkvT = [xkvT_pool.tile([128, K], BF16, name=f"xkvT{c}", tag=f"xkvT{c}")
            for c in range(nP)]
    for c in range(nP):
        tload(xkvT[c][:], xkv_d[:, ts(c, 128)])

    wT_pool = pool("wT", 1)
    wvT = [wT_pool.tile([128, D], BF16, name=f"wvT{c}", tag=f"w{c}")
           for c in range(nP)]
    for c in range(nP):
        tload(wvT[c][:], w_d[2 * D:3 * D, ts(c, 128)])

    vaug_pool = pool("vaug", 1)
    vaug = [vaug_pool.tile([128, H * (dh + 1)], BF16, name=f"vaug{kb}",
                           tag=f"vaug{kb}") for kb in range(nKB)]
    ECH = 384
    hpe = ECH // dh  # heads per output chunk
    for kb in range(nKB):
        va3 = vaug[kb].rearrange("p (h c) -> p h c", c=dh + 1)
        # whole-tile memset to 1.0; V copies below leave the ones columns
        nc.gpsimd.memset(vaug[kb][:], 1.0)
        for e in range(D // ECH):
            vp = psum.tile([128, ECH], F32, name="vp", tag="sc", bufs=2)
            for c in range(nP):
                nc.tensor.matmul(vp[:], xkvT[c][:, ts(kb, 128)],
                                 wvT[c][:, ts(e, ECH)],
                                 start=(c == 0), stop=(c == nP - 1))
            nc.vector.tensor_copy(
                va3[:, e * hpe:(e + 1) * hpe, 0:dh],
                vp[:].rearrange("p (h c) -> p h c", c=dh))

    # ---- K^T projection (scaled by 1/sqrt(dh)) ----
    wkT = [wT_pool.tile([128, D], BF16, name=f"wkT{c}", tag=f"w{c}")
           for c in range(nP)]
    for c in range(nP):
        tload(wkT[c][:], w_d[D:2 * D, ts(c, 128)])
    kT_pool = pool("kT", 1)
    kT = [kT_pool.tile([128, K], BF16, name=f"kT{p}", tag=f"kT{p}")
          for p in range(nP)]
    for p in range(nP):
        for j in range(K // 512):
            kp = psum.tile([128, 512], F32, name="kp", tag="sc", bufs=2)
            for c in range(nP):
                nc.tensor.matmul(kp[:], wkT[c][:, ts(p, 128)],
                                 xkvT[c][:, ts(j, 512)],
                                 start=(c == 0), stop=(c == nP - 1))
            nc.vector.tensor_scalar_mul(kT[p][:, ts(j, 512)], kp[:], scale)
    dump("kT0", kT[0][:])
    dump("vaug0", vaug[0][:])

    # ---- Q weights / proj weights, transposed in ----
    wqT = [wT_pool.tile([128, D], BF16, name=f"wqT{c}", tag=f"w{c}")
           for c in range(nP)]
    for c in range(nP):
        tload(wqT[c][:], w_d[0:D, ts(c, 128)])
    pwT_pool = pool("pwT", 1)
    pwT = [pwT_pool.tile([128, D], BF16, name=f"pwT{c}", tag=f"pwT{c}")
           for c in range(nP)]
    for c in range(nP):
        tload(pwT[c][:], pw_d[:, ts(c, 128)])

    # ---- bias broadcast [1, D] -> [128, D] (gpsimd) ----
    b_sb = misc.tile([1, D], F32, tag="b_sb")
    nc.sync.dma_start(b_sb[:], pb_d[:])
    b_bc = misc.tile([128, D], F32, tag="b_bc")
    nc.gpsimd.partition_broadcast(b_bc[:], b_sb[:])

    # ---- attention pools ----
    xqT_p = pool("xqT", 2)
    qT_p = pool("qT", nP + 1)
    expT_p = pool("expT", 4)
    outT_p = pool("outT", nP + 1)
    rinv_p = pool("rinv", 3)
    y_p = pool("y", 3)

    for qc in range(nQC):
        # Q^T projection for this chunk, in 512-token sub-chunks
        qT = [qT_p.tile([128, QCW], BF16, name=f"qT{p}", tag="qT")
              for p in range(nP)]
        for s in range(QCW // 512):
            xqT = [xqT_p.tile([128, 512], BF16, name=f"xqT{c}", tag=f"xqT{c}")
                   for c in range(nP)]
            for c in range(nP):
                row = qc * QCW + s * 512
                tload(xqT[c][:], xq_d[row:row + 512, ts(c, 128)])
            for p in range(nP):
                qp = psum.tile([128, 512], F32, name="qp", tag="sc", bufs=2)
                for c in range(nP):
                    nc.tensor.matmul(qp[:], wqT[c][:, ts(p, 128)], xqT[c][:],
                                     start=(c == 0), stop=(c == nP - 1))
                nc.vector.tensor_copy(qT[p][:, ts(s, 512)], qp[:])
        if qc == 0:
            dump("qT0", qT[0][:])

        # attention per head pair (2 heads row-packed in the PE array)
        for p in range(nP):
            outT = outT_p.tile([128, QCW], BF16, name="outT", tag="outT")
            av = [psum.tile([dh + 1, 512], F32, name="av", tag="av", bufs=4)
                  for _ in range(2 * (QCW // 512))]
            for kb in range(nKB):
                scs = [psum.tile([128, QCW], F32, name="sc", tag="sc",
                                 bufs=2) for _ in range(2)]
                for jn in range(QCW // 512):
                    for t in range(2):
                        nc.tensor.matmul(
                            scs[t][:, ts(jn, 512)],
                            kT[p][t * dh:(t + 1) * dh, ts(kb, 128)],
                            qT[p][t * dh:(t + 1) * dh, ts(jn, 512)],
                            start=True, stop=True)
                expT = []
                for t in range(2):
                    ex = expT_p.tile([128, QCW], BF16, name="ex", tag="expT")
                    nc.scalar.activation(ex[:], scs[t][:], EXP)
                    expT.append(ex)
                    if qc == 0 and p == 0 and kb == 0 and t == 0:
                        dump("ex00", ex[:])
                for t in range(2):
                    h = 2 * p + t
                    for s in range(QCW // 512):
                        nc.tensor.matmul(
                            av[t * (QCW // 512) + s][:],
                            vaug[kb][:, h * (dh + 1):(h + 1) * (dh + 1)],
                            expT[t][:, ts(s, 512)],
                            start=(kb == 0), stop=(kb == nKB - 1))
            for t in range(2):
                for s in range(QCW // 512):
                    a = av[t * (QCW // 512) + s]
                    if qc == 0 and p == 0 and t == 0 and s == 0:
                        dump("av00", a[:])
                    # custom-DVE ops ignore the input base partition; stage
                    # the denominator row to partition 0 first
                    den = rinv_p.tile([1, 512], F32, name="den", tag="den")
                    nc.vector.tensor_copy(den[:], a[dh:dh + 1, :])
                    rinv = rinv_p.tile([1, 512], F32, name="rinv", tag="rinv")
                    nc.vector.reciprocal_approx_fast(rinv[:], den[:])
                    bc = rinv_p.tile([dh, 512], F32, name="bc", tag="bc")
                    nc.gpsimd.partition_broadcast(bc[:], rinv[:])
                    if qc == 0 and p == 0 and t == 0 and s == 0:
                        dump("bc00", bc[:])
                    nc.vector.tensor_mul(
                        outT[t * dh:(t + 1) * dh, ts(s, 512)],
                        a[0:dh, :], bc[:])
            if p == 0:
                outTs = []
                if qc == 0:
                    dump("outT0", outT[:])
            outTs.append(outT)

        # output projection for this chunk
        for tb in range(QCW // 128):
            y_sb = y_p.tile([128, D], F32, name="y_sb", tag="y")
            for e in range(D // ECH):
                yp = psum.tile([128, ECH], F32, name="yp", tag="sc", bufs=2)
                for p in range(nP):
                    nc.tensor.matmul(yp[:], outTs[p][:, ts(tb, 128)],
                                     pwT[p][:, ts(e, ECH)],
                                     start=(p == 0), stop=(p == nP - 1))
                nc.vector.tensor_add(y_sb[:, ts(e, ECH)], yp[:],
                                     b_bc[:, ts(e, ECH)])
            row = qc * QCW + tb * 128
            nc.sync.dma_start(y_d[row:row + 128, :], y_sb[:])


def _get_program(S_loc, K, D, H, scale):
    key = (S_loc, K, D, H, scale)
    if key not in _cache:
        _cache[key] = _build(S_loc, K, D, H, scale)
    return _cache[key]


def _make_in_maps(x, qkv_w, proj_w, proj_b, stride):
    N, S, D = x.shape
    halves = N_CORES // N
    S_loc = S // halves
    bf = ml_dtypes.bfloat16
    w16 = np.ascontiguousarray(np.asarray(qkv_w, np.float32).astype(bf))
    pw16 = np.ascontiguousarray(np.asarray(proj_w, np.float32).astype(bf))
    pb2 = np.ascontiguousarray(np.asarray(proj_b, np.float32).reshape(1, D))
    x16 = np.asarray(x, np.float32).astype(bf)
    in_maps = []
    for c in range(N_CORES):
        n, hlf = c // halves, c % halves
        in_maps.append({
            "xq": np.ascontiguousarray(x16[n, hlf * S_loc:(hlf + 1) * S_loc]),
            "xkv": np.ascontiguousarray(x16[n, ::stride, :]),
            "qkv_w": w16,
            "proj_w": pw16,
            "proj_b": pb2,
        })
    return in_maps, S_loc, halves


def kernel(x, qkv_w, proj_w, proj_b, stride):
    x = np.asarray(x, dtype=np.float32)
    stride = int(stride)
    N, S, D = x.shape
    H = 12
    scale = float(D // H) ** -0.5
    K = S // stride

    in_maps, S_loc, halves = _make_in_maps(x, qkv_w, proj_w, proj_b, stride)
    nc = _get_program(S_loc, K, D, H, scale)
    res = bass_utils.run_bass_kernel_spmd(nc, in_maps,
                                          core_ids=list(range(N_CORES)))
    y = np.empty((N, S, D), dtype=np.float32)
    for c in range(N_CORES):
        n, hlf = c // halves, c % halves
        y[n, hlf * S_loc:(hlf + 1) * S_loc, :] = res.results[c]["y"]
    return y


def run_traced(x, qkv_w, proj_w, proj_b, stride, trace_cores=None):
    """test.py helper: same as kernel() but returns (y, BassKernelResults)."""
    import types
    try:
        import antenv.axon_hooks  # noqa: F401
    except ImportError:
        from trn_agent_boot.trn_boot import _ntff_profile_via_ctypes
        hook = _ntff_profile_via_ctypes("/opt/axon/libaxon_pjrt.so")
        mod = types.ModuleType("antenv.axon_hooks")
        mod.get_axon_ntff_profile_hook = lambda: hook
        mod.set_axon_ntff_profile_hook = lambda h: None
        sys.modules["antenv.axon_hooks"] = mod

    x = np.asarray(x, dtype=np.float32)
    stride = int(stride)
    N, S, D = x.shape
    H = 12
    scale = float(D // H) ** -0.5
    K = S // stride
    in_maps, S_loc, halves = _make_in_maps(x, qkv_w, proj_w, proj_b, stride)
    nc = _get_program(S_loc, K, D, H, scale)
    res = bass_utils.run_bass_kernel_spmd(
        nc, in_maps, core_ids=list(range(N_CORES)), trace=True,
        trace_cores=trace_cores or [0])
    y = np.empty((N, S, D), dtype=np.float32)
    for c in range(N_CORES):
        n, hlf = c // halves, c % halves
        y[n, hlf * S_loc:(hlf + 1) * S_loc, :] = res.results[c]["y"]
    return y, res
